# revision 22
# baseline (speedup 1.0000x reference)
"""Trainium2 Bass kernel for nn_FDModel_18433999634973.

The reference's attention pooling applies softmax over a singleton axis, so
the attention weights are identically 1.0 and each pooled embedding is just a
sum over the K axis.  The model therefore reduces to:

    p?   = sum_k X?[b, k, :]                      (for author/title/text)
    s?   = dot(p?, Wf?[0]) + bf?
    score  = sigmoid([sa, st, sx])                [B, 3]
    logits = score @ Wc.T + bc                    [B, 2]
    out    = softmax(logits, axis=1)

Sharding: pure data parallel over batch (512 -> 8 x 64).

The embeddings are cast to fp8 e3m4 on the host (4 mantissa bits): quarter
the fp32 HBM traffic at 1.38e-2 exact relative error on the seeded inputs
(verified by simulation; fp16 measures 1.98e-4, e4m3 2.9e-2 > tolerance).

At 1 byte/element the PE (1 row/cycle regardless of dtype; ~327 G elem/s
measured) would cap the kernel above the DMA floor, so 8 of the 32 text
chunks are offloaded to the otherwise idle VectorE (~133 G elem/s):
scalar_tensor_tensor multiplies the raw fp8 tile by a stride-0-broadcast
fp16 weight tile and accumulates the per-partition dot directly (accum_out);
the [128] partials are folded to [64] batch rows by one tiny f32 selector
matmul.  The remaining chunks flow through the selector-matmul k-sum on
TensorE, split over two PSUM tiles so the first dot overlaps the stream
tail.  The author/title sigmoids and their logit-difference contribution run
mid-stream; only the text-score chain (2 adds, sigmoid, one fused
multiply-add, 2 sigmoids) remains in the serial tail, and softmax over the
2 classes is computed as a sigmoid of the logit difference.

Latency trims: the small author stream leads so engines start ~2 us
earlier; the final text chunk lands as two half tiles so the PE drain after
the last DMA is halved; the closing chain is one fused 3-way add plus three
chained ACT sigmoids whose scale/bias ride per-partition APs (the
ddp-derived biases are precomputed mid-stream).

Measured (hardware-loop repeat-delta, 8 concurrent cores): ~97 us/exec vs
171.9 us for the fp16 selector-matmul baseline under the same metric; the
DMA-only floor for this traffic is ~85-87 us (~320 GB/s/core sustained;
multi-ring HWDGE splits and Pool-engine SWDGE do not raise it).  Exact
full-batch relative error 1.375e-2.
"""

import numpy as np
import ml_dtypes

import concourse.bacc as bacc
import concourse.mybir as mybir
import concourse.tile as tile
from concourse.bass_utils import run_bass_kernel_spmd

N_CORES = 8
B = 512
B_SH = B // N_CORES  # 64
KA, KT, KX = 8, 32, 512
DA, DS = 256, 768

# wpack column offsets
OFF_WFX = 0
OFF_WFT = DS
OFF_WFA = 2 * DS
OFF_WC0 = 2 * DS + DA
OFF_WC1 = OFF_WC0 + 3
OFF_B3 = OFF_WC1 + 3
OFF_BC = OFF_B3 + 3
OFF_Z8 = OFF_BC + 2  # eight host-zeroed columns; col 2 gets sx2, col 6 sx_dve
OFF_DWC = OFF_Z8 + 8  # Wc[0,j]-Wc[1,j] for j=0,1,2
OFF_NDWC2 = OFF_DWC + 3  # -(Wc[0,2]-Wc[1,2])
WPACK = OFF_NDWC2 + 1  # 1812

F32 = mybir.dt.float32
F16 = mybir.dt.float16
AL = mybir.AluOpType
ACT = mybir.ActivationFunctionType

MODE_DT = {
    "f8": mybir.dt.float8e3,
    "f16": mybir.dt.float16,
    "f32": mybir.dt.float32,
}
MODE_NP = {
    "f8": ml_dtypes.float8_e3m4,
    "f16": np.float16,
    "f32": np.float32,
}

# default knobs (overridable in build_module for experiments)
DEFAULT_MODE = "f8"
DEFAULT_DVE = 8  # of the 32 text chunks, how many go to VectorE
DEFAULT_GPS = 0  # ... and how many to the Pool engine (gpsimd)
DEFAULT_RINGS = ("sync",)  # HWDGE rings for the stream DMAs


def build_module(b_sh: int = B_SH, mm_mode: str = DEFAULT_MODE, repeat: int = 1,
                 n_dve: int = DEFAULT_DVE, n_gps: int = DEFAULT_GPS,
                 dma_only: bool = False, xt_bufs: int = 8, loop_n: int = 0,
                 rings: tuple = ("sync",)):
    MDT = MODE_DT[mm_mode]
    nc = bacc.Bacc(
        "TRN2",
        target_bir_lowering=False,
        debug=False,
        enable_asserts=True,
        num_devices=N_CORES,
    )
    ring_engs = [getattr(nc, r) for r in rings]
    ring_state = [0]

    def stream_dma(dst, src):
        eng = ring_engs[ring_state[0] % len(ring_engs)]
        ring_state[0] += 1
        eng.dma_start(dst, src)
    xt = nc.dram_tensor("xt", [b_sh, KX, DS], MDT, kind="ExternalInput")
    xs = nc.dram_tensor("xs", [b_sh, KT, DS], MDT, kind="ExternalInput")
    xa = nc.dram_tensor("xa", [b_sh, KA, DA], MDT, kind="ExternalInput")
    wpack = nc.dram_tensor("wpack", [b_sh, WPACK], F32, kind="ExternalInput")
    # selector: selg[p, p // KP] = 1
    GB = 64 if b_sh % 64 == 0 else 32  # batch rows per matmul group
    KP = 128 // GB  # k rows folded into the partition dim
    n_groups = b_sh // GB
    selg = nc.dram_tensor("selg", [128, GB], MDT, kind="ExternalInput")
    n_off = n_dve + n_gps
    if n_off:
        selgf = nc.dram_tensor("selgf", [128, GB], F32, kind="ExternalInput")
        wfx128 = nc.dram_tensor("wfx128", [128, DS], F16, kind="ExternalInput")
    out = nc.dram_tensor("out", [b_sh, 2], F32, kind="ExternalOutput")

    with tile.TileContext(nc) as tc:
        with (
            tc.tile_pool(name="consts", bufs=2) as consts,
            tc.tile_pool(name="xtp", bufs=xt_bufs) as xtp,
            tc.tile_pool(name="dvp", bufs=5) as dvp,
            tc.tile_pool(name="xsp", bufs=2) as xsp,
            tc.tile_pool(name="xap", bufs=2) as xap,
            tc.tile_pool(name="st2", bufs=1) as st2,
            tc.tile_pool(name="psum", bufs=1, space="PSUM") as psum,
        ):
          def emit_body():
            # consts ride the scalar engine's HWDGE ring, off the stream FIFO
            selg_t = consts.tile([128, GB], MDT)
            nc.scalar.dma_start(selg_t[:], selg.ap())
            wp = consts.tile([b_sh, WPACK], F32)
            nc.scalar.dma_start(wp[:], wpack.ap())
            if n_off:
                selgf_t = consts.tile([128, GB], F32)
                nc.scalar.dma_start(selgf_t[:], selgf.ap())
                wfx_t = consts.tile([128, DS], F16)
                nc.scalar.dma_start(wfx_t[:], wfx128.ap())
                dve_scratch = st2.tile([128, 8 * DS], F32)
                acc = st2.tile([128, max(n_off, 1)], F32)
            if n_gps:
                gps_scratch = st2.tile([128, 8 * DS], F32)

            ps_t = psum.tile([b_sh, DS], F32)
            ps_t2 = psum.tile([b_sh, DS], F32)
            ps_s = psum.tile([b_sh, DS], F32)
            ps_a = psum.tile([b_sh, DA], F32)
            if n_off:
                ps_dv = psum.tile([b_sh, max(n_off, 1)], F32)

            def reduce_stream(x_ap, K, D, ps_list, pool, dve_set=(), gps_set=()):
                """sum over k of x[b, k, :].  Chunks in dve_set/gps_set are
                handled by VectorE/PoolE (weighted partial dot into acc); the
                rest run selector matmuls on TensorE, split across the psum
                tiles in ps_list."""
                KR = K // KP  # k rows in the free/chunk dims
                CH = min(KR, 8)  # k rows per SBUF tile
                n_ch = KR // CH
                off = dve_set + gps_set
                pe_chunks = [c for c in range(n_ch) if c not in off]
                per = (len(pe_chunks) + len(ps_list) - 1) // len(ps_list)
                # PSUM-bank-aligned output slices (bank = 512 fp32)
                dhs = [(lo, min(D, lo + 512)) for lo in range(0, D, 512)]
                for g in range(n_groups):
                    x3 = x_ap[g * GB : (g + 1) * GB].rearrange(
                        "b (k0 kc k1) d -> (b k0) kc (k1 d)", k0=KP, k1=CH
                    )
                    for c in range(n_ch):
                        if dma_only:
                            t = pool.tile([128, CH * D], MDT)
                            stream_dma(t[:], x3[:, c, :])
                            continue
                        if c in off:
                            j = off.index(c)
                            eng = nc.vector if c in dve_set else nc.gpsimd
                            scr = dve_scratch if c in dve_set else gps_scratch
                            t = dvp.tile([128, CH * D], MDT)
                            stream_dma(t[:], x3[:, c, :])
                            in0 = t[:].rearrange("p (k d) -> p k d", k=CH)
                            in1 = wfx_t[:].unsqueeze(1).broadcast_to(
                                [128, CH, DS]
                            )
                            o3 = scr[:, : CH * DS].rearrange(
                                "p (k d) -> p k d", k=CH
                            )
                            eng.scalar_tensor_tensor(
                                out=o3,
                                in0=in0,
                                scalar=1.0,
                                in1=in1,
                                op0=AL.mult,
                                op1=AL.mult,
                                accum_out=acc[:, j : j + 1],
                            )
                            continue
                        i = pe_chunks.index(c)
                        ps_tile = ps_list[i // per]
                        first = (i % per) == 0
                        last = i == len(pe_chunks) - 1 or (i % per) == per - 1
                        stream_last = i == len(pe_chunks) - 1 and CH % 2 == 0
                        if stream_last:
                            # final chunk lands as two half tiles so the PE
                            # drain after the last DMA is halved
                            H = CH // 2
                            halves = []
                            for h in range(2):
                                th = pool.tile([128, H * D], MDT)
                                stream_dma(th[:], x3[:, c, h * H * D : (h + 1) * H * D])
                                halves.append(th)
                        else:
                            t = pool.tile([128, CH * D], MDT)
                            stream_dma(t[:], x3[:, c, :])
                        for k1 in range(CH):
                            if stream_last:
                                H = CH // 2
                                tt, koff = halves[k1 // H], k1 % H
                            else:
                                tt, koff = t, k1
                            for lo, hi in dhs:
                                nc.tensor.matmul(
                                    ps_tile[g * GB : (g + 1) * GB, lo:hi],
                                    selg_t[:],
                                    tt[:, koff * D + lo : koff * D + hi],
                                    start=(first and k1 == 0),
                                    stop=(last and k1 == CH - 1),
                                )

            # ---- stage 2 tiles ----
            scratch = st2.tile([b_sh, DS], F32)
            s3 = st2.tile([b_sh, 4], F32)
            z8 = wp[:, OFF_Z8 : OFF_Z8 + 8]  # pre-zeroed on the host
            score = st2.tile([b_sh, 4], F32)
            tmp1 = st2.tile([b_sh, 1], F32)
            ddp = st2.tile([b_sh, 1], F32)
            ddp0 = st2.tile([b_sh, 1], F32)
            ddp1 = st2.tile([b_sh, 1], F32)
            sxa = st2.tile([b_sh, 2], F32)
            outt = st2.tile([b_sh, 2], F32)

            def dot(ps_tile, w_lo, Dd, acc_ap):
                nc.vector.scalar_tensor_tensor(
                    out=scratch[:, 0:Dd],
                    in0=ps_tile[:, 0:Dd],
                    scalar=1.0,
                    in1=wp[:, w_lo : w_lo + Dd],
                    op0=AL.mult,
                    op1=AL.mult,
                    accum_out=acc_ap,
                )

            if dma_only:
                reduce_stream(xs.ap(), KT, DS, [ps_s], xsp)
                reduce_stream(xa.ap(), KA, DA, [ps_a], xap)
                reduce_stream(xt.ap(), KX, DS, [ps_t, ps_t2], xtp)
                nc.vector.memset(outt[:, 0:2], 0.0)
                nc.sync.dma_start(out.ap(), outt[:, 0:2])
                return

            # small streams first (author smallest, so engines start ~2us
            # sooner): their dot products run on the otherwise idle VectorE
            # while TensorE is still streaming text; the text stream itself is
            # split across two PSUM tiles so the first half's dot also leaves
            # the serial tail.
            reduce_stream(xa.ap(), KA, DA, [ps_a], xap)
            dot(ps_a, OFF_WFA, DA, s3[:, 0:1])
            reduce_stream(xs.ap(), KT, DS, [ps_s], xsp)
            dot(ps_s, OFF_WFT, DS, s3[:, 1:2])
            # Offloaded text chunks sit early/middle of the stream so VectorE
            # and PoolE keep pace with the DMA and finish before the tail.
            off_pos = [1 + 2 * j for j in range(n_off)]
            dve_set = tuple(off_pos[:n_dve])
            gps_set = tuple(off_pos[n_dve:])
            reduce_stream(xt.ap(), KX, DS, [ps_t, ps_t2], xtp, dve_set, gps_set)
            dot(ps_t, OFF_WFX, DS, s3[:, 2:3])
            # author/title sigmoids + their logit-difference contribution run
            # mid-stream on the otherwise idle ScalarE/VectorE
            nc.scalar.activation(
                score[:, 0:1], s3[:, 0:1], ACT.Sigmoid,
                bias=wp[:, OFF_B3 + 0 : OFF_B3 + 1], scale=1.0,
            )
            nc.scalar.activation(
                score[:, 1:2], s3[:, 1:2], ACT.Sigmoid,
                bias=wp[:, OFF_B3 + 1 : OFF_B3 + 2], scale=1.0,
            )
            nc.vector.tensor_scalar_mul(
                tmp1[:, 0:1], score[:, 1:2],
                wp[:, OFF_DWC + 1 : OFF_DWC + 2],
            )
            nc.vector.scalar_tensor_tensor(
                out=ddp[:, 0:1],
                in0=score[:, 0:1],
                scalar=wp[:, OFF_DWC + 0 : OFF_DWC + 1],
                in1=tmp1[:, 0:1],
                op0=AL.mult,
                op1=AL.add,
            )
            nc.vector.tensor_scalar_add(
                ddp0[:, 0:1], ddp[:, 0:1], wp[:, OFF_BC : OFF_BC + 1]
            )
            nc.vector.tensor_scalar(
                out=ddp1[:, 0:1], in0=ddp[:, 0:1],
                scalar1=-1.0, scalar2=wp[:, OFF_BC + 1 : OFF_BC + 2],
                op0=AL.mult, op1=AL.add,
            )
            dot(ps_t2, OFF_WFX, DS, z8[:, 2:3])
            if n_off:
                # fold [128] engine partials onto [64] batch rows, then reduce
                nc.tensor.matmul(
                    ps_dv[:, 0:n_off], selgf_t[:], acc[:, 0:n_off],
                    start=True, stop=True,
                )
                nc.vector.tensor_reduce(
                    z8[:, 6:7], ps_dv[:, 0:n_off], axis=mybir.AxisListType.X,
                    op=AL.add,
                )

            # ---- tail: only the text-score chain remains serial ----
            # softmax over 2 classes == sigmoid of the logit difference
            # dd = sum_j score_j * (Wc[0,j]-Wc[1,j]); author/title parts were
            # computed mid-stream into ddp.
            if n_off:
                nc.vector.scalar_tensor_tensor(
                    out=sxa[:, 0:1],
                    in0=s3[:, 2:3],
                    scalar=z8[:, 2:3],
                    in1=z8[:, 6:7],
                    op0=AL.add,
                    op1=AL.add,
                )
            else:
                nc.vector.tensor_tensor(
                    sxa[:, 0:1], s3[:, 2:3], z8[:, 2:3], op=AL.add
                )
            nc.scalar.activation(
                score[:, 2:3], sxa[:, 0:1], ACT.Sigmoid,
                bias=wp[:, OFF_B3 + 2 : OFF_B3 + 3], scale=1.0,
            )
            # out0 = sigmoid(dWc2*scoreX + ddp + dbc0),
            # out1 = sigmoid(-dWc2*scoreX - ddp + dbc1) -- both as one ACT op
            # each via per-partition scale/bias APs
            nc.scalar.activation(
                outt[:, 0:1], score[:, 2:3], ACT.Sigmoid,
                bias=ddp0[:, 0:1], scale=wp[:, OFF_DWC + 2 : OFF_DWC + 3],
            )
            nc.scalar.activation(
                outt[:, 1:2], score[:, 2:3], ACT.Sigmoid,
                bias=ddp1[:, 0:1], scale=wp[:, OFF_NDWC2 : OFF_NDWC2 + 1],
            )
            nc.sync.dma_start(out.ap(), outt[:, 0:2])

          if loop_n > 1:
              with tc.For_i(0, loop_n):
                  emit_body()
          else:
              for _rep in range(repeat):
                  emit_body()

    nc.compile()
    return nc


def make_host_inputs(Wfa, bfa, Wft, bft, Wfx, bfx, Wc, bc, b_sh: int = B_SH,
                     sel_np=None, n_dve: int = DEFAULT_DVE):
    """Build the replicated small-tensor inputs."""
    if sel_np is None:
        sel_np = MODE_NP[DEFAULT_MODE]
    wpack = np.zeros((WPACK,), np.float32)
    wpack[OFF_WFX : OFF_WFX + DS] = Wfx[0]
    wpack[OFF_WFT : OFF_WFT + DS] = Wft[0]
    wpack[OFF_WFA : OFF_WFA + DA] = Wfa[0]
    wpack[OFF_WC0 : OFF_WC0 + 3] = Wc[0]
    wpack[OFF_WC1 : OFF_WC1 + 3] = Wc[1]
    wpack[OFF_B3 + 0] = bfa[0]
    wpack[OFF_B3 + 1] = bft[0]
    wpack[OFF_B3 + 2] = bfx[0]
    wpack[OFF_BC + 0] = bc[0] - bc[1]
    wpack[OFF_BC + 1] = bc[1] - bc[0]
    wpack[OFF_DWC : OFF_DWC + 3] = Wc[0] - Wc[1]
    wpack[OFF_NDWC2] = Wc[1][2] - Wc[0][2]
    wpack_b = np.ascontiguousarray(np.broadcast_to(wpack, (b_sh, WPACK)))

    GB = 64 if b_sh % 64 == 0 else 32
    KP = 128 // GB
    p = np.arange(128)
    selg = np.zeros((128, GB), sel_np)
    selg[p, p // KP] = 1.0
    extras = {}
    if n_dve:
        selgf = np.zeros((128, GB), np.float32)
        selgf[p, p // KP] = 1.0
        extras["selgf"] = selgf
        extras["wfx128"] = np.ascontiguousarray(
            np.broadcast_to(Wfx[0].astype(np.float16), (128, DS))
        )
    return wpack_b, selg, extras


_NC_CACHE = {}


def kernel(author_emb, title_emb, text_emb,
           Wa, ba, ca, Wt, bt, ct, Wx, bx, cx,
           Wfa, bfa, Wft, bft, Wfx, bfx, Wc, bc):
    key = "full"
    if key not in _NC_CACHE:
        _NC_CACHE[key] = build_module(B_SH, mm_mode=DEFAULT_MODE,
                                      n_dve=DEFAULT_DVE, n_gps=DEFAULT_GPS,
                                      rings=DEFAULT_RINGS)
    nc = _NC_CACHE[key]

    np_dt = MODE_NP[DEFAULT_MODE]
    author_emb = np.asarray(author_emb, np.float32).astype(np_dt)
    title_emb = np.asarray(title_emb, np.float32).astype(np_dt)
    text_emb = np.asarray(text_emb, np.float32).astype(np_dt)
    wpack_b, selg, extras = make_host_inputs(
        np.asarray(Wfa), np.asarray(bfa), np.asarray(Wft), np.asarray(bft),
        np.asarray(Wfx), np.asarray(bfx), np.asarray(Wc), np.asarray(bc),
        sel_np=np_dt, n_dve=DEFAULT_DVE + DEFAULT_GPS,
    )

    in_maps = []
    for c in range(N_CORES):
        sl = slice(c * B_SH, (c + 1) * B_SH)
        in_maps.append(
            {
                "xt": np.ascontiguousarray(text_emb[sl]),
                "xs": np.ascontiguousarray(title_emb[sl]),
                "xa": np.ascontiguousarray(author_emb[sl]),
                "wpack": wpack_b,
                "selg": selg,
                **extras,
            }
        )

    res = run_bass_kernel_spmd(nc, in_maps, core_ids=list(range(N_CORES)))
    return np.concatenate([res.results[c]["out"] for c in range(N_CORES)], axis=0)


# revision 23
# speedup vs baseline: 1.0044x; 1.0044x over previous
"""Trainium2 Bass kernel for nn_FDModel_18433999634973.

The reference's attention pooling applies softmax over a singleton axis, so
the attention weights are identically 1.0 and each pooled embedding is just a
sum over the K axis.  The model therefore reduces to:

    p?   = sum_k X?[b, k, :]                      (for author/title/text)
    s?   = dot(p?, Wf?[0]) + bf?
    score  = sigmoid([sa, st, sx])                [B, 3]
    logits = score @ Wc.T + bc                    [B, 2]
    out    = softmax(logits, axis=1)

Sharding: pure data parallel over batch (512 -> 8 x 64).

The embeddings are cast to fp8 e3m4 on the host (4 mantissa bits): quarter
the fp32 HBM traffic at 1.38e-2 exact relative error on the seeded inputs
(verified by simulation; fp16 measures 1.98e-4, e4m3 2.9e-2 > tolerance).

At 1 byte/element the PE (1 row/cycle regardless of dtype; ~327 G elem/s
measured) would cap the kernel above the DMA floor, so 8 of the 32 text
chunks are offloaded to the otherwise idle VectorE (~133 G elem/s):
scalar_tensor_tensor multiplies the raw fp8 tile by a stride-0-broadcast
fp16 weight tile and accumulates the per-partition dot directly (accum_out);
the [128] partials are folded to [64] batch rows by one tiny f32 selector
matmul.  The remaining chunks flow through the selector-matmul k-sum on
TensorE, split over two PSUM tiles so the first dot overlaps the stream
tail.  The author/title sigmoids and their logit-difference contribution run
mid-stream; only the text-score chain (2 adds, sigmoid, one fused
multiply-add, 2 sigmoids) remains in the serial tail, and softmax over the
2 classes is computed as a sigmoid of the logit difference.

Latency trims: the small author stream leads so engines start ~2 us
earlier; the final text chunk lands as two half tiles so the PE drain after
the last DMA is halved; the closing chain is one fused 3-way add plus three
chained ACT sigmoids whose scale/bias ride per-partition APs (the
ddp-derived biases are precomputed mid-stream).

Measured (hardware-loop repeat-delta, 8 concurrent cores): ~97 us/exec vs
171.9 us for the fp16 selector-matmul baseline under the same metric; the
DMA-only floor for this traffic is ~85-87 us (~320 GB/s/core sustained;
multi-ring HWDGE splits and Pool-engine SWDGE do not raise it).  Exact
full-batch relative error 1.375e-2.
"""

import numpy as np
import ml_dtypes

import concourse.bacc as bacc
import concourse.mybir as mybir
import concourse.tile as tile
from concourse.bass_utils import run_bass_kernel_spmd

N_CORES = 8
B = 512
B_SH = B // N_CORES  # 64
KA, KT, KX = 8, 32, 512
DA, DS = 256, 768

# weight rows live in a separate f16 tensor (wpw); wpack keeps the small
# f32 scalars (biases, device-written z columns, logit-diff weights)
OFF_WFX = 0
OFF_WFT = DS
OFF_WFA = 2 * DS
WPW = 2 * DS + DA  # 1792
OFF_B3 = 0
OFF_BC = OFF_B3 + 3
OFF_Z8 = OFF_BC + 2  # eight host-zeroed columns; col 2 gets sx2, col 6 sx_dve
OFF_DWC = OFF_Z8 + 8  # Wc[0,j]-Wc[1,j] for j=0,1,2
OFF_NDWC2 = OFF_DWC + 3  # -(Wc[0,2]-Wc[1,2])
WPACK = OFF_NDWC2 + 1  # 17

F32 = mybir.dt.float32
F16 = mybir.dt.float16
AL = mybir.AluOpType
ACT = mybir.ActivationFunctionType

MODE_DT = {
    "f8": mybir.dt.float8e3,
    "f16": mybir.dt.float16,
    "f32": mybir.dt.float32,
}
MODE_NP = {
    "f8": ml_dtypes.float8_e3m4,
    "f16": np.float16,
    "f32": np.float32,
}

# default knobs (overridable in build_module for experiments)
DEFAULT_MODE = "f8"
DEFAULT_DVE = 8  # of the 32 text chunks, how many go to VectorE
DEFAULT_GPS = 0  # ... and how many to the Pool engine (gpsimd)
DEFAULT_RINGS = ("sync",)  # HWDGE rings for the stream DMAs


def build_module(b_sh: int = B_SH, mm_mode: str = DEFAULT_MODE, repeat: int = 1,
                 n_dve: int = DEFAULT_DVE, n_gps: int = DEFAULT_GPS,
                 dma_only: bool = False, xt_bufs: int = 8, loop_n: int = 0,
                 rings: tuple = ("sync",)):
    MDT = MODE_DT[mm_mode]
    nc = bacc.Bacc(
        "TRN2",
        target_bir_lowering=False,
        debug=False,
        enable_asserts=True,
        num_devices=N_CORES,
    )
    ring_engs = [getattr(nc, r) for r in rings]
    ring_state = [0]

    def stream_dma(dst, src):
        eng = ring_engs[ring_state[0] % len(ring_engs)]
        ring_state[0] += 1
        eng.dma_start(dst, src)
    xt = nc.dram_tensor("xt", [b_sh, KX, DS], MDT, kind="ExternalInput")
    xs = nc.dram_tensor("xs", [b_sh, KT, DS], MDT, kind="ExternalInput")
    xa = nc.dram_tensor("xa", [b_sh, KA, DA], MDT, kind="ExternalInput")
    wpack = nc.dram_tensor("wpack", [b_sh, WPACK], F32, kind="ExternalInput")
    wpw = nc.dram_tensor("wpw", [b_sh, WPW], F16, kind="ExternalInput")
    # selector: selg[p, p // KP] = 1
    GB = 64 if b_sh % 64 == 0 else 32  # batch rows per matmul group
    KP = 128 // GB  # k rows folded into the partition dim
    n_groups = b_sh // GB
    selg = nc.dram_tensor("selg", [128, GB], MDT, kind="ExternalInput")
    n_off = n_dve + n_gps
    if n_off:
        selgf = nc.dram_tensor("selgf", [128, GB], F32, kind="ExternalInput")
        wfx128 = nc.dram_tensor("wfx128", [128, DS], F16, kind="ExternalInput")
    out = nc.dram_tensor("out", [b_sh, 2], F32, kind="ExternalOutput")

    with tile.TileContext(nc) as tc:
        with (
            tc.tile_pool(name="consts", bufs=2) as consts,
            tc.tile_pool(name="xtp", bufs=xt_bufs) as xtp,
            tc.tile_pool(name="dvp", bufs=5) as dvp,
            tc.tile_pool(name="xsp", bufs=2) as xsp,
            tc.tile_pool(name="xap", bufs=2) as xap,
            tc.tile_pool(name="st2", bufs=1) as st2,
            tc.tile_pool(name="psum", bufs=1, space="PSUM") as psum,
        ):
          def emit_body():
            # consts ride the scalar engine's HWDGE ring, off the stream FIFO
            selg_t = consts.tile([128, GB], MDT)
            nc.scalar.dma_start(selg_t[:], selg.ap())
            wp = consts.tile([b_sh, WPACK], F32)
            nc.scalar.dma_start(wp[:], wpack.ap())
            wpw_t = consts.tile([b_sh, WPW], F16)
            nc.scalar.dma_start(wpw_t[:], wpw.ap())
            if n_off:
                selgf_t = consts.tile([128, GB], F32)
                nc.scalar.dma_start(selgf_t[:], selgf.ap())
                wfx_t = consts.tile([128, DS], F16)
                nc.scalar.dma_start(wfx_t[:], wfx128.ap())
                dve_scratch = st2.tile([128, 8 * DS], F32)
                acc = st2.tile([128, max(n_off, 1)], F32)
            if n_gps:
                gps_scratch = st2.tile([128, 8 * DS], F32)

            ps_t = psum.tile([b_sh, DS], F32)
            ps_t2 = psum.tile([b_sh, DS], F32)
            ps_s = psum.tile([b_sh, DS], F32)
            ps_a = psum.tile([b_sh, DA], F32)
            if n_off:
                ps_dv = psum.tile([b_sh, max(n_off, 1)], F32)

            def reduce_stream(x_ap, K, D, ps_list, pool, dve_set=(), gps_set=()):
                """sum over k of x[b, k, :].  Chunks in dve_set/gps_set are
                handled by VectorE/PoolE (weighted partial dot into acc); the
                rest run selector matmuls on TensorE, split across the psum
                tiles in ps_list."""
                KR = K // KP  # k rows in the free/chunk dims
                CH = min(KR, 8)  # k rows per SBUF tile
                n_ch = KR // CH
                off = dve_set + gps_set
                pe_chunks = [c for c in range(n_ch) if c not in off]
                per = (len(pe_chunks) + len(ps_list) - 1) // len(ps_list)
                # PSUM-bank-aligned output slices (bank = 512 fp32)
                dhs = [(lo, min(D, lo + 512)) for lo in range(0, D, 512)]
                for g in range(n_groups):
                    x3 = x_ap[g * GB : (g + 1) * GB].rearrange(
                        "b (k0 kc k1) d -> (b k0) kc (k1 d)", k0=KP, k1=CH
                    )
                    for c in range(n_ch):
                        if dma_only:
                            t = pool.tile([128, CH * D], MDT)
                            stream_dma(t[:], x3[:, c, :])
                            continue
                        if c in off:
                            j = off.index(c)
                            eng = nc.vector if c in dve_set else nc.gpsimd
                            scr = dve_scratch if c in dve_set else gps_scratch
                            t = dvp.tile([128, CH * D], MDT)
                            stream_dma(t[:], x3[:, c, :])
                            in0 = t[:].rearrange("p (k d) -> p k d", k=CH)
                            in1 = wfx_t[:].unsqueeze(1).broadcast_to(
                                [128, CH, DS]
                            )
                            o3 = scr[:, : CH * DS].rearrange(
                                "p (k d) -> p k d", k=CH
                            )
                            eng.scalar_tensor_tensor(
                                out=o3,
                                in0=in0,
                                scalar=1.0,
                                in1=in1,
                                op0=AL.mult,
                                op1=AL.mult,
                                accum_out=acc[:, j : j + 1],
                            )
                            continue
                        i = pe_chunks.index(c)
                        ps_tile = ps_list[i // per]
                        first = (i % per) == 0
                        last = i == len(pe_chunks) - 1 or (i % per) == per - 1
                        stream_last = i == len(pe_chunks) - 1 and CH % 2 == 0
                        if stream_last:
                            # final chunk lands as two half tiles so the PE
                            # drain after the last DMA is halved
                            H = CH // 2
                            halves = []
                            for h in range(2):
                                th = pool.tile([128, H * D], MDT)
                                stream_dma(th[:], x3[:, c, h * H * D : (h + 1) * H * D])
                                halves.append(th)
                        else:
                            t = pool.tile([128, CH * D], MDT)
                            stream_dma(t[:], x3[:, c, :])
                        for k1 in range(CH):
                            if stream_last:
                                H = CH // 2
                                tt, koff = halves[k1 // H], k1 % H
                            else:
                                tt, koff = t, k1
                            for lo, hi in dhs:
                                nc.tensor.matmul(
                                    ps_tile[g * GB : (g + 1) * GB, lo:hi],
                                    selg_t[:],
                                    tt[:, koff * D + lo : koff * D + hi],
                                    start=(first and k1 == 0),
                                    stop=(last and k1 == CH - 1),
                                )

            # ---- stage 2 tiles ----
            scratch = st2.tile([b_sh, DS], F32)
            s3 = st2.tile([b_sh, 4], F32)
            z8 = wp[:, OFF_Z8 : OFF_Z8 + 8]  # pre-zeroed on the host
            score = st2.tile([b_sh, 4], F32)
            tmp1 = st2.tile([b_sh, 1], F32)
            ddp = st2.tile([b_sh, 1], F32)
            ddp0 = st2.tile([b_sh, 1], F32)
            ddp1 = st2.tile([b_sh, 1], F32)
            sxa = st2.tile([b_sh, 2], F32)
            outt = st2.tile([b_sh, 2], F32)

            def dot(ps_tile, w_lo, Dd, acc_ap):
                nc.vector.scalar_tensor_tensor(
                    out=scratch[:, 0:Dd],
                    in0=ps_tile[:, 0:Dd],
                    scalar=1.0,
                    in1=wpw_t[:, w_lo : w_lo + Dd],
                    op0=AL.mult,
                    op1=AL.mult,
                    accum_out=acc_ap,
                )

            if dma_only:
                reduce_stream(xs.ap(), KT, DS, [ps_s], xsp)
                reduce_stream(xa.ap(), KA, DA, [ps_a], xap)
                reduce_stream(xt.ap(), KX, DS, [ps_t, ps_t2], xtp)
                nc.vector.memset(outt[:, 0:2], 0.0)
                nc.sync.dma_start(out.ap(), outt[:, 0:2])
                return

            # small streams first (author smallest, so engines start ~2us
            # sooner): their dot products run on the otherwise idle VectorE
            # while TensorE is still streaming text; the text stream itself is
            # split across two PSUM tiles so the first half's dot also leaves
            # the serial tail.
            reduce_stream(xa.ap(), KA, DA, [ps_a], xap)
            dot(ps_a, OFF_WFA, DA, s3[:, 0:1])
            reduce_stream(xs.ap(), KT, DS, [ps_s], xsp)
            dot(ps_s, OFF_WFT, DS, s3[:, 1:2])
            # Offloaded text chunks sit early/middle of the stream so VectorE
            # and PoolE keep pace with the DMA and finish before the tail.
            off_pos = [1 + 2 * j for j in range(n_off)]
            dve_set = tuple(off_pos[:n_dve])
            gps_set = tuple(off_pos[n_dve:])
            reduce_stream(xt.ap(), KX, DS, [ps_t, ps_t2], xtp, dve_set, gps_set)
            dot(ps_t, OFF_WFX, DS, s3[:, 2:3])
            # author/title sigmoids + their logit-difference contribution run
            # mid-stream on the otherwise idle ScalarE/VectorE
            nc.scalar.activation(
                score[:, 0:1], s3[:, 0:1], ACT.Sigmoid,
                bias=wp[:, OFF_B3 + 0 : OFF_B3 + 1], scale=1.0,
            )
            nc.scalar.activation(
                score[:, 1:2], s3[:, 1:2], ACT.Sigmoid,
                bias=wp[:, OFF_B3 + 1 : OFF_B3 + 2], scale=1.0,
            )
            nc.vector.tensor_scalar_mul(
                tmp1[:, 0:1], score[:, 1:2],
                wp[:, OFF_DWC + 1 : OFF_DWC + 2],
            )
            nc.vector.scalar_tensor_tensor(
                out=ddp[:, 0:1],
                in0=score[:, 0:1],
                scalar=wp[:, OFF_DWC + 0 : OFF_DWC + 1],
                in1=tmp1[:, 0:1],
                op0=AL.mult,
                op1=AL.add,
            )
            nc.vector.tensor_scalar_add(
                ddp0[:, 0:1], ddp[:, 0:1], wp[:, OFF_BC : OFF_BC + 1]
            )
            nc.vector.tensor_scalar(
                out=ddp1[:, 0:1], in0=ddp[:, 0:1],
                scalar1=-1.0, scalar2=wp[:, OFF_BC + 1 : OFF_BC + 2],
                op0=AL.mult, op1=AL.add,
            )
            dot(ps_t2, OFF_WFX, DS, z8[:, 2:3])
            if n_off:
                # fold [128] engine partials onto [64] batch rows, then reduce
                nc.tensor.matmul(
                    ps_dv[:, 0:n_off], selgf_t[:], acc[:, 0:n_off],
                    start=True, stop=True,
                )
                nc.vector.tensor_reduce(
                    z8[:, 6:7], ps_dv[:, 0:n_off], axis=mybir.AxisListType.X,
                    op=AL.add,
                )

            # ---- tail: only the text-score chain remains serial ----
            # softmax over 2 classes == sigmoid of the logit difference
            # dd = sum_j score_j * (Wc[0,j]-Wc[1,j]); author/title parts were
            # computed mid-stream into ddp.
            if n_off:
                nc.vector.scalar_tensor_tensor(
                    out=sxa[:, 0:1],
                    in0=s3[:, 2:3],
                    scalar=z8[:, 2:3],
                    in1=z8[:, 6:7],
                    op0=AL.add,
                    op1=AL.add,
                )
            else:
                nc.vector.tensor_tensor(
                    sxa[:, 0:1], s3[:, 2:3], z8[:, 2:3], op=AL.add
                )
            nc.scalar.activation(
                score[:, 2:3], sxa[:, 0:1], ACT.Sigmoid,
                bias=wp[:, OFF_B3 + 2 : OFF_B3 + 3], scale=1.0,
            )
            # out0 = sigmoid(dWc2*scoreX + ddp + dbc0),
            # out1 = sigmoid(-dWc2*scoreX - ddp + dbc1) -- both as one ACT op
            # each via per-partition scale/bias APs
            nc.scalar.activation(
                outt[:, 0:1], score[:, 2:3], ACT.Sigmoid,
                bias=ddp0[:, 0:1], scale=wp[:, OFF_DWC + 2 : OFF_DWC + 3],
            )
            nc.scalar.activation(
                outt[:, 1:2], score[:, 2:3], ACT.Sigmoid,
                bias=ddp1[:, 0:1], scale=wp[:, OFF_NDWC2 : OFF_NDWC2 + 1],
            )
            nc.sync.dma_start(out.ap(), outt[:, 0:2])

          if loop_n > 1:
              with tc.For_i(0, loop_n):
                  emit_body()
          else:
              for _rep in range(repeat):
                  emit_body()

    nc.compile()
    return nc


def make_host_inputs(Wfa, bfa, Wft, bft, Wfx, bfx, Wc, bc, b_sh: int = B_SH,
                     sel_np=None, n_dve: int = DEFAULT_DVE):
    """Build the replicated small-tensor inputs."""
    if sel_np is None:
        sel_np = MODE_NP[DEFAULT_MODE]
    wpw = np.zeros((WPW,), np.float16)
    wpw[OFF_WFX : OFF_WFX + DS] = Wfx[0].astype(np.float16)
    wpw[OFF_WFT : OFF_WFT + DS] = Wft[0].astype(np.float16)
    wpw[OFF_WFA : OFF_WFA + DA] = Wfa[0].astype(np.float16)
    wpack = np.zeros((WPACK,), np.float32)
    wpack[OFF_B3 + 0] = bfa[0]
    wpack[OFF_B3 + 1] = bft[0]
    wpack[OFF_B3 + 2] = bfx[0]
    wpack[OFF_BC + 0] = bc[0] - bc[1]
    wpack[OFF_BC + 1] = bc[1] - bc[0]
    wpack[OFF_DWC : OFF_DWC + 3] = Wc[0] - Wc[1]
    wpack[OFF_NDWC2] = Wc[1][2] - Wc[0][2]
    wpack_b = np.ascontiguousarray(np.broadcast_to(wpack, (b_sh, WPACK)))
    wpw_b = np.ascontiguousarray(np.broadcast_to(wpw, (b_sh, WPW)))

    GB = 64 if b_sh % 64 == 0 else 32
    KP = 128 // GB
    p = np.arange(128)
    selg = np.zeros((128, GB), sel_np)
    selg[p, p // KP] = 1.0
    extras = {"wpw": wpw_b}
    if n_dve:
        selgf = np.zeros((128, GB), np.float32)
        selgf[p, p // KP] = 1.0
        extras["selgf"] = selgf
        extras["wfx128"] = np.ascontiguousarray(
            np.broadcast_to(Wfx[0].astype(np.float16), (128, DS))
        )
    return wpack_b, selg, extras


_NC_CACHE = {}


def kernel(author_emb, title_emb, text_emb,
           Wa, ba, ca, Wt, bt, ct, Wx, bx, cx,
           Wfa, bfa, Wft, bft, Wfx, bfx, Wc, bc):
    key = "full"
    if key not in _NC_CACHE:
        _NC_CACHE[key] = build_module(B_SH, mm_mode=DEFAULT_MODE,
                                      n_dve=DEFAULT_DVE, n_gps=DEFAULT_GPS,
                                      rings=DEFAULT_RINGS)
    nc = _NC_CACHE[key]

    np_dt = MODE_NP[DEFAULT_MODE]
    author_emb = np.asarray(author_emb, np.float32).astype(np_dt)
    title_emb = np.asarray(title_emb, np.float32).astype(np_dt)
    text_emb = np.asarray(text_emb, np.float32).astype(np_dt)
    wpack_b, selg, extras = make_host_inputs(
        np.asarray(Wfa), np.asarray(bfa), np.asarray(Wft), np.asarray(bft),
        np.asarray(Wfx), np.asarray(bfx), np.asarray(Wc), np.asarray(bc),
        sel_np=np_dt, n_dve=DEFAULT_DVE + DEFAULT_GPS,
    )

    in_maps = []
    for c in range(N_CORES):
        sl = slice(c * B_SH, (c + 1) * B_SH)
        in_maps.append(
            {
                "xt": np.ascontiguousarray(text_emb[sl]),
                "xs": np.ascontiguousarray(title_emb[sl]),
                "xa": np.ascontiguousarray(author_emb[sl]),
                "wpack": wpack_b,
                "selg": selg,
                **extras,
            }
        )

    res = run_bass_kernel_spmd(nc, in_maps, core_ids=list(range(N_CORES)))
    return np.concatenate([res.results[c]["out"] for c in range(N_CORES)], axis=0)


# revision 26
# speedup vs baseline: 1.0447x; 1.0401x over previous
"""Trainium2 Bass kernel for nn_FDModel_18433999634973.

The reference's attention pooling applies softmax over a singleton axis, so
the attention weights are identically 1.0 and each pooled embedding is just a
sum over the K axis.  The model therefore reduces to:

    p?   = sum_k X?[b, k, :]                      (for author/title/text)
    s?   = dot(p?, Wf?[0]) + bf?
    score  = sigmoid([sa, st, sx])                [B, 3]
    logits = score @ Wc.T + bc                    [B, 2]
    out    = softmax(logits, axis=1)

Sharding: pure data parallel over batch (512 -> 8 x 64).

The embeddings are cast to fp8 e3m4 on the host (4 mantissa bits): quarter
the fp32 HBM traffic at 1.38e-2 exact relative error on the seeded inputs
(verified by simulation; fp16 measures 1.98e-4, e4m3 2.9e-2 > tolerance).

At 1 byte/element the PE (1 row/cycle regardless of dtype; ~327 G elem/s
measured) would cap the kernel above the DMA floor, so 8 of the 32 text
chunks are offloaded to the otherwise idle VectorE (~133 G elem/s):
scalar_tensor_tensor multiplies the raw fp8 tile by a stride-0-broadcast
fp16 weight tile and accumulates the per-partition dot directly (accum_out);
the [128] partials are folded to [64] batch rows by one tiny f32 selector
matmul.  The remaining chunks flow through the selector-matmul k-sum on
TensorE, split over two PSUM tiles so the first dot overlaps the stream
tail.  The author/title sigmoids and their logit-difference contribution run
mid-stream; only the text-score chain (2 adds, sigmoid, one fused
multiply-add, 2 sigmoids) remains in the serial tail, and softmax over the
2 classes is computed as a sigmoid of the logit difference.

Latency trims: the small author stream leads so engines start ~2 us
earlier; the final text chunk lands as two half tiles so the PE drain after
the last DMA is halved; the closing chain is one fused 3-way add plus three
chained ACT sigmoids whose scale/bias ride per-partition APs (the
ddp-derived biases are precomputed mid-stream).

The dot weights ride a separate f16 const tensor (wpw) so the replicated
constant traffic is ~230 KB lighter; only the small f32 scalars (biases,
device-written z columns, logit-diff weights) stay replicated in fp32.

DVE chunks sit at every-4th stream position so their arrival rate matches
VectorE's consumption rate -- packing them denser exhausts the DVE tile pool
and the next DVE DMA head-of-line-blocks the FIFO queue, stalling the PE
stream (~3 us).

Measured (hardware-loop repeat-delta, 8 concurrent cores): 94.7 us/exec vs
171.9 us for the fp16 selector-matmul baseline under the same metric; the
DMA-only floor for this traffic is ~85-87 us (~320 GB/s/core sustained;
multi-ring HWDGE splits and Pool-engine SWDGE do not raise it).  Exact
full-batch relative error 1.388e-2.
"""

import numpy as np
import ml_dtypes

import concourse.bacc as bacc
import concourse.mybir as mybir
import concourse.tile as tile
from concourse.bass_utils import run_bass_kernel_spmd

N_CORES = 8
B = 512
B_SH = B // N_CORES  # 64
KA, KT, KX = 8, 32, 512
DA, DS = 256, 768

# weight rows live in a separate f16 tensor (wpw); wpack keeps the small
# f32 scalars (biases, device-written z columns, logit-diff weights)
OFF_WFX = 0
OFF_WFT = DS
OFF_WFA = 2 * DS
WPW = 2 * DS + DA  # 1792
OFF_B3 = 0
OFF_BC = OFF_B3 + 3
OFF_Z8 = OFF_BC + 2  # eight host-zeroed columns; col 2 gets sx2, col 6 sx_dve
OFF_DWC = OFF_Z8 + 8  # Wc[0,j]-Wc[1,j] for j=0,1,2
OFF_NDWC2 = OFF_DWC + 3  # -(Wc[0,2]-Wc[1,2])
WPACK = OFF_NDWC2 + 1  # 17

F32 = mybir.dt.float32
F16 = mybir.dt.float16
AL = mybir.AluOpType
ACT = mybir.ActivationFunctionType

MODE_DT = {
    "f8": mybir.dt.float8e3,
    "f16": mybir.dt.float16,
    "f32": mybir.dt.float32,
}
MODE_NP = {
    "f8": ml_dtypes.float8_e3m4,
    "f16": np.float16,
    "f32": np.float32,
}

# default knobs (overridable in build_module for experiments)
DEFAULT_MODE = "f8"
DEFAULT_DVE = 8  # of the 32 text chunks, how many go to VectorE
DEFAULT_GPS = 0  # ... and how many to the Pool engine (gpsimd)
DEFAULT_RINGS = ("sync",)  # HWDGE rings for the stream DMAs


def build_module(b_sh: int = B_SH, mm_mode: str = DEFAULT_MODE, repeat: int = 1,
                 n_dve: int = DEFAULT_DVE, n_gps: int = DEFAULT_GPS,
                 dma_only: bool = False, xt_bufs: int = 8, loop_n: int = 0,
                 rings: tuple = ("sync",), dve_stride: int = 4):
    MDT = MODE_DT[mm_mode]
    nc = bacc.Bacc(
        "TRN2",
        target_bir_lowering=False,
        debug=False,
        enable_asserts=True,
        num_devices=N_CORES,
    )
    ring_engs = [getattr(nc, r) for r in rings]
    ring_state = [0]

    def stream_dma(dst, src):
        eng = ring_engs[ring_state[0] % len(ring_engs)]
        ring_state[0] += 1
        eng.dma_start(dst, src)
    xt = nc.dram_tensor("xt", [b_sh, KX, DS], MDT, kind="ExternalInput")
    xs = nc.dram_tensor("xs", [b_sh, KT, DS], MDT, kind="ExternalInput")
    xa = nc.dram_tensor("xa", [b_sh, KA, DA], MDT, kind="ExternalInput")
    wpack = nc.dram_tensor("wpack", [b_sh, WPACK], F32, kind="ExternalInput")
    wpw = nc.dram_tensor("wpw", [b_sh, WPW], F16, kind="ExternalInput")
    # selector: selg[p, p // KP] = 1
    GB = 64 if b_sh % 64 == 0 else 32  # batch rows per matmul group
    KP = 128 // GB  # k rows folded into the partition dim
    n_groups = b_sh // GB
    selg = nc.dram_tensor("selg", [128, GB], MDT, kind="ExternalInput")
    n_off = n_dve + n_gps
    if n_off:
        selgf = nc.dram_tensor("selgf", [128, GB], F32, kind="ExternalInput")
        wfx128 = nc.dram_tensor("wfx128", [128, DS], F16, kind="ExternalInput")
    out = nc.dram_tensor("out", [b_sh, 2], F32, kind="ExternalOutput")

    with tile.TileContext(nc) as tc:
        with (
            tc.tile_pool(name="consts", bufs=2) as consts,
            tc.tile_pool(name="xtp", bufs=xt_bufs) as xtp,
            tc.tile_pool(name="dvp", bufs=5) as dvp,
            tc.tile_pool(name="xsp", bufs=2) as xsp,
            tc.tile_pool(name="xap", bufs=2) as xap,
            tc.tile_pool(name="st2", bufs=1) as st2,
            tc.tile_pool(name="psum", bufs=1, space="PSUM") as psum,
        ):
          def emit_body():
            # consts ride the scalar engine's HWDGE ring, off the stream FIFO
            selg_t = consts.tile([128, GB], MDT)
            nc.scalar.dma_start(selg_t[:], selg.ap())
            wp = consts.tile([b_sh, WPACK], F32)
            nc.scalar.dma_start(wp[:], wpack.ap())
            wpw_t = consts.tile([b_sh, WPW], F16)
            nc.scalar.dma_start(wpw_t[:], wpw.ap())
            if n_off:
                selgf_t = consts.tile([128, GB], F32)
                nc.scalar.dma_start(selgf_t[:], selgf.ap())
                wfx_t = consts.tile([128, DS], F16)
                nc.scalar.dma_start(wfx_t[:], wfx128.ap())
                dve_scratch = st2.tile([128, 8 * DS], F32)
                acc = st2.tile([128, max(n_off, 1)], F32)
            if n_gps:
                gps_scratch = st2.tile([128, 8 * DS], F32)

            ps_t = psum.tile([b_sh, DS], F32)
            ps_t2 = psum.tile([b_sh, DS], F32)
            ps_s = psum.tile([b_sh, DS], F32)
            ps_a = psum.tile([b_sh, DA], F32)
            if n_off:
                ps_dv = psum.tile([b_sh, max(n_off, 1)], F32)

            def reduce_stream(x_ap, K, D, ps_list, pool, dve_set=(), gps_set=()):
                """sum over k of x[b, k, :].  Chunks in dve_set/gps_set are
                handled by VectorE/PoolE (weighted partial dot into acc); the
                rest run selector matmuls on TensorE, split across the psum
                tiles in ps_list."""
                KR = K // KP  # k rows in the free/chunk dims
                CH = min(KR, 8)  # k rows per SBUF tile
                n_ch = KR // CH
                off = dve_set + gps_set
                pe_chunks = [c for c in range(n_ch) if c not in off]
                per = (len(pe_chunks) + len(ps_list) - 1) // len(ps_list)
                # PSUM-bank-aligned output slices (bank = 512 fp32)
                dhs = [(lo, min(D, lo + 512)) for lo in range(0, D, 512)]
                for g in range(n_groups):
                    x3 = x_ap[g * GB : (g + 1) * GB].rearrange(
                        "b (k0 kc k1) d -> (b k0) kc (k1 d)", k0=KP, k1=CH
                    )
                    for c in range(n_ch):
                        if dma_only:
                            t = pool.tile([128, CH * D], MDT)
                            stream_dma(t[:], x3[:, c, :])
                            continue
                        if c in off:
                            j = off.index(c)
                            eng = nc.vector if c in dve_set else nc.gpsimd
                            scr = dve_scratch if c in dve_set else gps_scratch
                            t = dvp.tile([128, CH * D], MDT)
                            stream_dma(t[:], x3[:, c, :])
                            in0 = t[:].rearrange("p (k d) -> p k d", k=CH)
                            in1 = wfx_t[:].unsqueeze(1).broadcast_to(
                                [128, CH, DS]
                            )
                            o3 = scr[:, : CH * DS].rearrange(
                                "p (k d) -> p k d", k=CH
                            )
                            eng.scalar_tensor_tensor(
                                out=o3,
                                in0=in0,
                                scalar=1.0,
                                in1=in1,
                                op0=AL.mult,
                                op1=AL.mult,
                                accum_out=acc[:, j : j + 1],
                            )
                            continue
                        i = pe_chunks.index(c)
                        ps_tile = ps_list[i // per]
                        first = (i % per) == 0
                        last = i == len(pe_chunks) - 1 or (i % per) == per - 1
                        stream_last = i == len(pe_chunks) - 1 and CH % 2 == 0
                        if stream_last:
                            # final chunk lands as two half tiles so the PE
                            # drain after the last DMA is halved
                            H = CH // 2
                            halves = []
                            for h in range(2):
                                th = pool.tile([128, H * D], MDT)
                                stream_dma(th[:], x3[:, c, h * H * D : (h + 1) * H * D])
                                halves.append(th)
                        else:
                            t = pool.tile([128, CH * D], MDT)
                            stream_dma(t[:], x3[:, c, :])
                        for k1 in range(CH):
                            if stream_last:
                                H = CH // 2
                                tt, koff = halves[k1 // H], k1 % H
                            else:
                                tt, koff = t, k1
                            for lo, hi in dhs:
                                nc.tensor.matmul(
                                    ps_tile[g * GB : (g + 1) * GB, lo:hi],
                                    selg_t[:],
                                    tt[:, koff * D + lo : koff * D + hi],
                                    start=(first and k1 == 0),
                                    stop=(last and k1 == CH - 1),
                                )

            # ---- stage 2 tiles ----
            scratch = st2.tile([b_sh, DS], F32)
            s3 = st2.tile([b_sh, 4], F32)
            z8 = wp[:, OFF_Z8 : OFF_Z8 + 8]  # pre-zeroed on the host
            score = st2.tile([b_sh, 4], F32)
            tmp1 = st2.tile([b_sh, 1], F32)
            ddp = st2.tile([b_sh, 1], F32)
            ddp0 = st2.tile([b_sh, 1], F32)
            ddp1 = st2.tile([b_sh, 1], F32)
            sxa = st2.tile([b_sh, 2], F32)
            outt = st2.tile([b_sh, 2], F32)

            def dot(ps_tile, w_lo, Dd, acc_ap):
                nc.vector.scalar_tensor_tensor(
                    out=scratch[:, 0:Dd],
                    in0=ps_tile[:, 0:Dd],
                    scalar=1.0,
                    in1=wpw_t[:, w_lo : w_lo + Dd],
                    op0=AL.mult,
                    op1=AL.mult,
                    accum_out=acc_ap,
                )

            if dma_only:
                reduce_stream(xs.ap(), KT, DS, [ps_s], xsp)
                reduce_stream(xa.ap(), KA, DA, [ps_a], xap)
                reduce_stream(xt.ap(), KX, DS, [ps_t, ps_t2], xtp)
                nc.vector.memset(outt[:, 0:2], 0.0)
                nc.sync.dma_start(out.ap(), outt[:, 0:2])
                return

            # small streams first (author smallest, so engines start ~2us
            # sooner): their dot products run on the otherwise idle VectorE
            # while TensorE is still streaming text; the text stream itself is
            # split across two PSUM tiles so the first half's dot also leaves
            # the serial tail.
            reduce_stream(xa.ap(), KA, DA, [ps_a], xap)
            dot(ps_a, OFF_WFA, DA, s3[:, 0:1])
            reduce_stream(xs.ap(), KT, DS, [ps_s], xsp)
            dot(ps_s, OFF_WFT, DS, s3[:, 1:2])
            # Offloaded text chunks sit early/middle of the stream so VectorE
            # and PoolE keep pace with the DMA and finish before the tail.
            off_pos = [1 + dve_stride * j for j in range(n_off)]
            dve_set = tuple(off_pos[:n_dve])
            gps_set = tuple(off_pos[n_dve:])
            reduce_stream(xt.ap(), KX, DS, [ps_t, ps_t2], xtp, dve_set, gps_set)
            dot(ps_t, OFF_WFX, DS, s3[:, 2:3])
            # author/title sigmoids + their logit-difference contribution run
            # mid-stream on the otherwise idle ScalarE/VectorE
            nc.scalar.activation(
                score[:, 0:1], s3[:, 0:1], ACT.Sigmoid,
                bias=wp[:, OFF_B3 + 0 : OFF_B3 + 1], scale=1.0,
            )
            nc.scalar.activation(
                score[:, 1:2], s3[:, 1:2], ACT.Sigmoid,
                bias=wp[:, OFF_B3 + 1 : OFF_B3 + 2], scale=1.0,
            )
            nc.vector.tensor_scalar_mul(
                tmp1[:, 0:1], score[:, 1:2],
                wp[:, OFF_DWC + 1 : OFF_DWC + 2],
            )
            nc.vector.scalar_tensor_tensor(
                out=ddp[:, 0:1],
                in0=score[:, 0:1],
                scalar=wp[:, OFF_DWC + 0 : OFF_DWC + 1],
                in1=tmp1[:, 0:1],
                op0=AL.mult,
                op1=AL.add,
            )
            nc.vector.tensor_scalar_add(
                ddp0[:, 0:1], ddp[:, 0:1], wp[:, OFF_BC : OFF_BC + 1]
            )
            nc.vector.tensor_scalar(
                out=ddp1[:, 0:1], in0=ddp[:, 0:1],
                scalar1=-1.0, scalar2=wp[:, OFF_BC + 1 : OFF_BC + 2],
                op0=AL.mult, op1=AL.add,
            )
            dot(ps_t2, OFF_WFX, DS, z8[:, 2:3])
            if n_off:
                # fold [128] engine partials onto [64] batch rows, then reduce
                nc.tensor.matmul(
                    ps_dv[:, 0:n_off], selgf_t[:], acc[:, 0:n_off],
                    start=True, stop=True,
                )
                nc.vector.tensor_reduce(
                    z8[:, 6:7], ps_dv[:, 0:n_off], axis=mybir.AxisListType.X,
                    op=AL.add,
                )

            # ---- tail: only the text-score chain remains serial ----
            # softmax over 2 classes == sigmoid of the logit difference
            # dd = sum_j score_j * (Wc[0,j]-Wc[1,j]); author/title parts were
            # computed mid-stream into ddp.
            if n_off:
                nc.vector.scalar_tensor_tensor(
                    out=sxa[:, 0:1],
                    in0=s3[:, 2:3],
                    scalar=z8[:, 2:3],
                    in1=z8[:, 6:7],
                    op0=AL.add,
                    op1=AL.add,
                )
            else:
                nc.vector.tensor_tensor(
                    sxa[:, 0:1], s3[:, 2:3], z8[:, 2:3], op=AL.add
                )
            nc.scalar.activation(
                score[:, 2:3], sxa[:, 0:1], ACT.Sigmoid,
                bias=wp[:, OFF_B3 + 2 : OFF_B3 + 3], scale=1.0,
            )
            # out0 = sigmoid(dWc2*scoreX + ddp + dbc0),
            # out1 = sigmoid(-dWc2*scoreX - ddp + dbc1) -- both as one ACT op
            # each via per-partition scale/bias APs
            nc.scalar.activation(
                outt[:, 0:1], score[:, 2:3], ACT.Sigmoid,
                bias=ddp0[:, 0:1], scale=wp[:, OFF_DWC + 2 : OFF_DWC + 3],
            )
            nc.scalar.activation(
                outt[:, 1:2], score[:, 2:3], ACT.Sigmoid,
                bias=ddp1[:, 0:1], scale=wp[:, OFF_NDWC2 : OFF_NDWC2 + 1],
            )
            nc.sync.dma_start(out.ap(), outt[:, 0:2])

          if loop_n > 1:
              with tc.For_i(0, loop_n):
                  emit_body()
          else:
              for _rep in range(repeat):
                  emit_body()

    nc.compile()
    return nc


def make_host_inputs(Wfa, bfa, Wft, bft, Wfx, bfx, Wc, bc, b_sh: int = B_SH,
                     sel_np=None, n_dve: int = DEFAULT_DVE):
    """Build the replicated small-tensor inputs."""
    if sel_np is None:
        sel_np = MODE_NP[DEFAULT_MODE]
    wpw = np.zeros((WPW,), np.float16)
    wpw[OFF_WFX : OFF_WFX + DS] = Wfx[0].astype(np.float16)
    wpw[OFF_WFT : OFF_WFT + DS] = Wft[0].astype(np.float16)
    wpw[OFF_WFA : OFF_WFA + DA] = Wfa[0].astype(np.float16)
    wpack = np.zeros((WPACK,), np.float32)
    wpack[OFF_B3 + 0] = bfa[0]
    wpack[OFF_B3 + 1] = bft[0]
    wpack[OFF_B3 + 2] = bfx[0]
    wpack[OFF_BC + 0] = bc[0] - bc[1]
    wpack[OFF_BC + 1] = bc[1] - bc[0]
    wpack[OFF_DWC : OFF_DWC + 3] = Wc[0] - Wc[1]
    wpack[OFF_NDWC2] = Wc[1][2] - Wc[0][2]
    wpack_b = np.ascontiguousarray(np.broadcast_to(wpack, (b_sh, WPACK)))
    wpw_b = np.ascontiguousarray(np.broadcast_to(wpw, (b_sh, WPW)))

    GB = 64 if b_sh % 64 == 0 else 32
    KP = 128 // GB
    p = np.arange(128)
    selg = np.zeros((128, GB), sel_np)
    selg[p, p // KP] = 1.0
    extras = {"wpw": wpw_b}
    if n_dve:
        selgf = np.zeros((128, GB), np.float32)
        selgf[p, p // KP] = 1.0
        extras["selgf"] = selgf
        extras["wfx128"] = np.ascontiguousarray(
            np.broadcast_to(Wfx[0].astype(np.float16), (128, DS))
        )
    return wpack_b, selg, extras


_NC_CACHE = {}


def kernel(author_emb, title_emb, text_emb,
           Wa, ba, ca, Wt, bt, ct, Wx, bx, cx,
           Wfa, bfa, Wft, bft, Wfx, bfx, Wc, bc):
    key = "full"
    if key not in _NC_CACHE:
        _NC_CACHE[key] = build_module(B_SH, mm_mode=DEFAULT_MODE,
                                      n_dve=DEFAULT_DVE, n_gps=DEFAULT_GPS,
                                      rings=DEFAULT_RINGS)
    nc = _NC_CACHE[key]

    np_dt = MODE_NP[DEFAULT_MODE]
    author_emb = np.asarray(author_emb, np.float32).astype(np_dt)
    title_emb = np.asarray(title_emb, np.float32).astype(np_dt)
    text_emb = np.asarray(text_emb, np.float32).astype(np_dt)
    wpack_b, selg, extras = make_host_inputs(
        np.asarray(Wfa), np.asarray(bfa), np.asarray(Wft), np.asarray(bft),
        np.asarray(Wfx), np.asarray(bfx), np.asarray(Wc), np.asarray(bc),
        sel_np=np_dt, n_dve=DEFAULT_DVE + DEFAULT_GPS,
    )

    in_maps = []
    for c in range(N_CORES):
        sl = slice(c * B_SH, (c + 1) * B_SH)
        in_maps.append(
            {
                "xt": np.ascontiguousarray(text_emb[sl]),
                "xs": np.ascontiguousarray(title_emb[sl]),
                "xa": np.ascontiguousarray(author_emb[sl]),
                "wpack": wpack_b,
                "selg": selg,
                **extras,
            }
        )

    res = run_bass_kernel_spmd(nc, in_maps, core_ids=list(range(N_CORES)))
    return np.concatenate([res.results[c]["out"] for c in range(N_CORES)], axis=0)
